# revision 33
# baseline (speedup 1.0000x reference)
"""Trainium2 Bass kernel for a GQA attention block (B=1, T=2048, C=4096,
NH=32, NKV=8, HS=128), tensor-parallel over heads across 8 NeuronCores.

Per core c: 4 query heads (4c..4c+3) and 1 KV head (c). All matmul
operands are fp16 (full PE speed, FWL weight loads, half the HBM traffic
of fp32, 8x finer mantissa than bf16); PSUM accumulation stays fp32.

  - projections computed DIRECTLY in transposed layout: lhsT = W chunk
    (stationary), rhs = x^T chunk (moving, 512 wide)  ->  qT/kT/vT [HS, T]
  - c-chunk-major loop + engine-split DMA queues (x^T on GpSimd, weights
    on SP): first matmul only needs 1/32 of the weights, compute starts
    a few us in instead of waiting for the full weight DMA
  - RoPE on transposed tiles: rot = R @ qT via one PE matmul (R = exact
    +-1 rotate-half matrix), then qkT = qT*cosT + rot*sinT on DVE
  - V transposed back to natural [T, HS] via PE transpose (needed as the
    stationary operand of the P@V matmul)
  - attention on S^T blocks [keys, queries] with causally-reduced widths;
    exp batched in kc-pairs [128,1024] with bias -4 (keeps p in fp16
    range); softmax denominator via an all-ones matmul (broadcast over
    all 128 partitions for free); 1/l = exp(-ln l) on ScalarE (both in
    the natural_log_exp table set) instead of the slow DVE reciprocal
  - c_proj partial interleaved with the next query block's attention,
    drains split between DVE and ACT; partials summed on host (the TP
    all-reduce), fp16 on the wire.
"""
import sys

sys.path.insert(0, "/opt/trn_rl_repo")

import numpy as np
import ml_dtypes

from contextlib import ExitStack

import concourse.bass as bass
import concourse.mybir as mybir
import concourse.tile as tile
from concourse.bass_utils import run_bass_kernel_spmd

# ---------------------------------------------------------------- constants
B, T, C = 1, 2048, 4096
NH, NKV, HS = 32, 8, 128
NCORES = 8
QH = NH // NCORES          # 4 query heads per core
DQ = QH * HS               # 512
NKC = C // 128             # 32 contraction chunks
NTB = T // 512             # 4 T-blocks (= query blocks)
NTM = T // 128             # 16 T-chunks
BASE, SCALE = 10000.0, 1.0
INV_SQRT_HS = 1.0 / float(np.sqrt(HS))
EXP_BIAS = -4.0            # exp(s/sqrt(HS) - 4): scale-invariant in softmax,
                           # keeps p well inside fp16 range

F32 = mybir.dt.float32
F16 = mybir.dt.float16
BF16 = mybir.dt.bfloat16

# ------------------------------------------------------- wait legalization
_TAIL_RUNWAY = 48


def _legalize_waits(nc):
    """walrus (this toolchain) allows ONE sync wait per ISA instruction.
    Split excess waits off onto standalone EventSemaphore instructions
    inserted immediately before the offender (same engine stream order)."""
    n_split = 0
    for bb in nc.m.functions[0].blocks:
        insts = bb.instructions
        if not any(i.sync_info and i.sync_info.on_wait and
                   len(i.sync_info.on_wait) > (0 if type(i).__name__ == "InstISA" else 1)
                   for i in insts):
            continue
        new_list = []
        for inst in insts:
            si = inst.sync_info
            is_raw_isa = type(inst).__name__ == "InstISA"
            keep_n = 0 if is_raw_isa else 1
            if si and si.on_wait and len(si.on_wait) > keep_n:
                waits = list(si.on_wait)
                split_off = waits if is_raw_isa else waits[:-1]
                for w in split_off:
                    ev = mybir.InstNoOp(
                        name=f"legal-wait-{nc.next_id()}",
                        ins=[], outs=[], engine=inst.engine,
                        bass_nofuse=True,
                        sync_info=mybir.SyncInfo(on_wait=[w], on_update=[]))
                    nc.register_instruction(ev, overwrite=True)
                    new_list.append(ev)
                    n_split += 1
                inst.sync_info = mybir.SyncInfo(
                    on_wait=[] if is_raw_isa else [waits[-1]],
                    on_update=list(si.on_update))
            new_list.append(inst)
        bb.instructions = new_list
    return n_split


def _audit(nc):
    bad = []
    for bb in nc.m.functions[0].blocks:
        for inst in bb.instructions:
            si = inst.sync_info
            if si and si.on_wait and len(si.on_wait) > 1:
                bad.append((type(inst).__name__, inst.name, str(inst.engine),
                            len(si.on_wait)))
    return bad


class _TailRunwayPatch:
    """Plant runway nops on SP right before Tile's tail drain so the drain's
    many queue waits can be redistributed by _legalize_waits."""

    def __enter__(self):
        self.orig = tile.TileContext._drain_and_barrier
        orig = self.orig

        def patched(tc_self, tick_clock, wait_clock):
            for _ in range(_TAIL_RUNWAY):
                tc_self.nc.sync.nop(nofuse=True)
            return orig(tc_self, tick_clock, wait_clock)

        tile.TileContext._drain_and_barrier = patched
        return self

    def __exit__(self, *a):
        tile.TileContext._drain_and_barrier = self.orig


# ---------------------------------------------------------------- builder

def _build_nc():
    nc = bass.Bass(trn_type="TRN2")

    xt = nc.dram_tensor("xt", [C, T], F16, kind="ExternalInput")
    wqkv = nc.dram_tensor("wqkv", [C, DQ + 2 * HS], F16, kind="ExternalInput")
    wct = nc.dram_tensor("wct", [DQ, C], F16, kind="ExternalInput")
    cosT = nc.dram_tensor("cosT", [HS, T], F16, kind="ExternalInput")
    sinT = nc.dram_tensor("sinT", [HS, T], F16, kind="ExternalInput")
    tri = nc.dram_tensor("tri", [128, 128], BF16, kind="ExternalInput")
    ones = nc.dram_tensor("ones", [128, 128], BF16, kind="ExternalInput")
    ident = nc.dram_tensor("ident", [128, 128], BF16, kind="ExternalInput")
    rmat = nc.dram_tensor("rmat", [128, 128], F16, kind="ExternalInput")
    bcol = nc.dram_tensor("bcol", [128, QH + 2], F32, kind="ExternalInput")
    out = nc.dram_tensor("out", [T, C], F16, kind="ExternalOutput")

    with _TailRunwayPatch(), tile.TileContext(nc) as tc:
        _trace_body(nc, tc, xt, wqkv, wct, cosT, sinT, tri, ones, ident,
                    rmat, bcol, out)

    _legalize_waits(nc)
    bad = _audit(nc)
    if bad:
        raise RuntimeError(f"multi-wait instructions remain: {bad[:10]}")
    return nc


def _trace_body(nc, tc, xt, wqkv, wct, cosT, sinT, tri, ones, ident, rmat,
                bcol, out):
    Ident = mybir.ActivationFunctionType.Identity
    Exp = mybir.ActivationFunctionType.Exp
    Ln = mybir.ActivationFunctionType.Ln

    persist = ExitStack()
    misc = persist.enter_context(tc.tile_pool(name="misc", bufs=1))
    qkt_pool = persist.enter_context(tc.tile_pool(name="qkt", bufs=1))
    v_pool = persist.enter_context(tc.tile_pool(name="vsb", bufs=1))

    tri_sb = misc.tile([128, 128], BF16)
    ones_sb = misc.tile([128, 128], BF16)
    ident_sb = misc.tile([128, 128], BF16)
    rmat_sb = misc.tile([128, 128], F16)
    bcol_sb = misc.tile([128, QH + 2], F32)

    qT = qkt_pool.tile([128, QH, T], F16)      # per-head q^T [HS, T], RoPE'd
    kT = qkt_pool.tile([128, T], F16)          # k^T [HS, T], RoPE'd
    v_sb = v_pool.tile([128, NTM, HS], BF16)    # V natural [T, HS]

    # ================= phase 1: projections + RoPE (transposed) ==========
    ph1 = ExitStack()
    w_pool = ph1.enter_context(tc.tile_pool(name="wqkv", bufs=1))
    xt_pool = ph1.enter_context(tc.tile_pool(name="xt", bufs=3))
    cs_pool = ph1.enter_context(tc.tile_pool(name="cossin", bufs=1))
    sc_pool = ph1.enter_context(tc.tile_pool(name="ropescr", bufs=4))
    tmp_pool = ph1.enter_context(tc.tile_pool(name="ropetmp", bufs=4))
    vt_pool = ph1.enter_context(tc.tile_pool(name="vtscr", bufs=3))
    ps_pr = ph1.enter_context(tc.tile_pool(name="pspr", bufs=6, space="PSUM"))
    ps_rot = ph1.enter_context(tc.tile_pool(name="psrot", bufs=2, space="PSUM"))

    w0_pool = ph1.enter_context(tc.tile_pool(name="w0", bufs=1))
    wqkv_sb = w_pool.tile([128, NKC, DQ + 2 * HS], F16)
    # x^T tiles ride the (otherwise idle) GpSimd queue; weights ride SP.
    # Interleave the first T-block's x chunks with the weight chunks so the
    # projection's c-chunk-major loop can start after ~2 chunks have landed.
    # Chunk 0 gets its own tiny tiles so the very first matmul has a precise
    # two-DMA dependency instead of a coarse multi-chunk semaphore threshold.
    w0_sb = w0_pool.tile([128, DQ + 2 * HS], F16)
    nc.sync.dma_start(out=w0_sb, in_=wqkv[0:128, :])
    x0_sb = w0_pool.tile([128, 512], F16)
    nc.gpsimd.dma_start(out=x0_sb, in_=xt[0:128, 0:512])
    xt_tiles = [None] * NTB
    xt_tiles[0] = xt_pool.tile([128, NKC, 512], F16, tag="xt", name="xt_sb")
    for kc in range(1, NKC):
        nc.sync.dma_start(out=wqkv_sb[:, kc, :],
                          in_=wqkv[kc * 128:(kc + 1) * 128, :])
        nc.gpsimd.dma_start(out=xt_tiles[0][:, kc, :],
                            in_=xt[kc * 128:(kc + 1) * 128, 0:512])
    nc.sync.dma_start(out=wqkv_sb[:, 0, :], in_=wqkv[0:128, :])
    cosT_sb = cs_pool.tile([128, T], F16)
    nc.scalar.dma_start(out=cosT_sb, in_=cosT[:, :])
    sinT_sb = cs_pool.tile([128, T], F16)
    nc.scalar.dma_start(out=sinT_sb, in_=sinT[:, :])
    nc.scalar.dma_start(out=rmat_sb, in_=rmat[:, :])
    nc.scalar.dma_start(out=bcol_sb, in_=bcol[:, :])
    nc.scalar.dma_start(out=tri_sb, in_=tri[:, :])
    nc.scalar.dma_start(out=ones_sb, in_=ones[:, :])
    nc.scalar.dma_start(out=ident_sb, in_=ident[:, :])

    NOC = QH + 2  # out chunks: q0..q3, k, v

    for tb in range(NTB):
        t0 = tb * 512
        if xt_tiles[tb] is None:
            xt_tiles[tb] = xt_pool.tile([128, NKC, 512], F16, tag="xt", name="xt_sb")
            for kc in range(NKC):
                nc.gpsimd.dma_start(out=xt_tiles[tb][:, kc, :],
                                    in_=xt[kc * 128:(kc + 1) * 128,
                                           t0:t0 + 512])
        xt_sb = xt_tiles[tb]
        # 6 accumulators [128, 512], one psum bank each
        ps = []
        for _oc in range(NOC):
            pr_acc = ps_pr.tile([128, 512], F32, tag="pr")
            ps.append(pr_acc)
        for kc in range(NKC):
            first = (tb == 0 and kc == 0)
            wsrc = w0_sb if first else wqkv_sb[:, kc, :]
            xsrc = x0_sb if first else xt_sb[:, kc, :]
            for oc in range(NOC):
                nc.tensor.matmul(ps[oc], wsrc[:, oc * 128:(oc + 1) * 128]
                                 if first else
                                 wqkv_sb[:, kc, oc * 128:(oc + 1) * 128],
                                 xsrc,
                                 start=(kc == 0), stop=(kc == NKC - 1),
                                 skip_group_check=True)

        # ---- RoPE for q0..q3 and k (oc = 0..QH) ----
        for oc in range(QH + 1):
            src = sc_pool.tile([128, 512], F16, tag="scr")
            if oc < QH:
                nc.scalar.activation(out=src, in_=ps[oc], func=Ident,
                                     bias=bcol_sb[:, oc:oc + 1], scale=1.0)
            else:
                nc.scalar.copy(out=src, in_=ps[oc])  # k: bias dropped (softmax-invariant)
            rot = ps_rot.tile([128, 512], F32, tag="rot")
            nc.tensor.matmul(rot, rmat_sb, src, start=True, stop=True,
                             skip_group_check=True)
            a = tmp_pool.tile([128, 512], F16, tag="tmp")
            nc.vector.tensor_mul(a, src, cosT_sb[:, t0:t0 + 512])
            b = tmp_pool.tile([128, 512], F16, tag="tmp")
            nc.vector.tensor_mul(b, rot, sinT_sb[:, t0:t0 + 512])
            dst = qT[:, oc, t0:t0 + 512] if oc < QH else kT[:, t0:t0 + 512]
            nc.vector.tensor_add(dst, a, b)

        # ---- V: drain (bias) then transpose to natural [T, HS] ----
        vt_sb = vt_pool.tile([128, 512], BF16)
        nc.scalar.activation(out=vt_sb, in_=ps[QH + 1], func=Ident,
                             bias=bcol_sb[:, QH:QH + 1], scale=1.0)
        vtr = ps_rot.tile([128, 512], BF16, tag="rot")
        for j in range(4):
            nc.tensor.matmul(vtr[:, j * 128:(j + 1) * 128],
                             vt_sb[:, j * 128:(j + 1) * 128], ident_sb,
                             is_transpose=True, skip_group_check=True)
        nc.vector.tensor_copy(out=v_sb[:, 4 * tb:4 * tb + 4, :], in_=vtr)

    ph1.close()

    # ============== phase 2: attention + c_proj, interleaved =============
    ph2 = ExitStack()
    wc_pool = ph2.enter_context(tc.tile_pool(name="wct", bufs=1))
    yt_pool = ph2.enter_context(tc.tile_pool(name="yt", bufs=1))
    pt_pool = ph2.enter_context(tc.tile_pool(name="pt", bufs=6))
    ls_pool = ph2.enter_context(tc.tile_pool(name="lsum", bufs=3))
    lw_pool = ph2.enter_context(tc.tile_pool(name="lwork", bufs=3))
    out_pool = ph2.enter_context(tc.tile_pool(name="outsb", bufs=8))
    ps_s = ph2.enter_context(tc.tile_pool(name="pss", bufs=2, space="PSUM"))
    ps_y = ph2.enter_context(tc.tile_pool(name="psy", bufs=1, space="PSUM"))
    ps_l = ph2.enter_context(tc.tile_pool(name="psl", bufs=1, space="PSUM"))
    ps_o = ph2.enter_context(tc.tile_pool(name="pso", bufs=2, space="PSUM"))

    wct_sb = wc_pool.tile([128, QH, C], F16)
    for h in range(QH):
        nc.sync.dma_start(out=wct_sb[:, h, :],
                          in_=wct[h * 128:(h + 1) * 128, :])
    yT = yt_pool.tile([128, QH, T], F16)

    def _attn_epilogue(h, qb, y_ps, l_ps):
        # 1/l = exp(-ln l) on ACT (exp & ln share one table set); the slow
        # DVE reciprocal serialized the whole pipeline.
        lnl = lw_pool.tile([128, 512], F32, tag="lw")
        nc.scalar.activation(out=lnl, in_=l_ps, func=Ln, scale=1.0)
        linv = lw_pool.tile([128, 512], F32, tag="lw")
        nc.scalar.activation(out=linv, in_=lnl, func=Exp, scale=-1.0)
        nc.vector.tensor_mul(yT[:, h, qb * 512:(qb + 1) * 512], y_ps, linv)

    def _cproj_tm(tm, tail=False):
        for oc in range(8):
            o_ps = ps_o.tile([128, 512], F32)
            for h in range(QH):
                nc.tensor.matmul(o_ps, yT[:, h, tm * 128:(tm + 1) * 128],
                                 wct_sb[:, h, oc * 512:(oc + 1) * 512],
                                 start=(h == 0), stop=(h == QH - 1),
                                 skip_group_check=True)
            out_sb = out_pool.tile([128, 512], F16)
            if (oc % 2 == 0) if tail else (oc < 6):
                nc.vector.tensor_copy(out=out_sb, in_=o_ps)
            else:
                nc.scalar.copy(out=out_sb, in_=o_ps)
            dst = out[tm * 128:(tm + 1) * 128, oc * 512:(oc + 1) * 512]
            if tail and oc % 2 == 1:
                nc.sync.dma_start(out=dst, in_=out_sb)
            else:
                nc.gpsimd.dma_start(out=dst, in_=out_sb)

    pending = None
    for qb in range(NTB):
        for h in range(QH):
            if pending is not None:
                _attn_epilogue(*pending)
                pending = None
            nkc = 4 * (qb + 1)
            y_ps = ps_y.tile([128, 512], F32)
            l_ps = ps_l.tile([128, 512], F32)
            prev = None  # previous kc-pair, its y/l matmuls emitted after
                         # the next pair's exp (software skew)

            lsum = None

            def _emit_y(pair):
                for (kc, off, w, pt, pcol) in pair:
                    nc.tensor.matmul(y_ps[:, off:512], v_sb[:, kc, :],
                                     pt[:, pcol:pcol + w], start=(kc == 0),
                                     stop=(kc == nkc - 1),
                                     skip_group_check=True)

            for p0 in range(0, nkc, 2):
                s_big = ps_s.tile([128, 1024], F32, tag="s")
                pt = pt_pool.tile([128, 1024], BF16, tag="pt")
                pair = []
                cols = 0
                for kc in (p0, p0 + 1):
                    o = kc - 4 * qb          # >=0 on diagonal blocks
                    off = max(o, 0) * 128    # first live query column
                    w = 512 - off
                    nc.tensor.matmul(s_big[:, cols:cols + w],
                                     kT[:, kc * 128:(kc + 1) * 128],
                                     qT[:, h, qb * 512 + off:(qb + 1) * 512],
                                     start=True, stop=True,
                                     skip_group_check=True)
                    pair.append((kc, off, w, pt, cols))
                    cols += w
                # one exp over both packed blocks
                nc.scalar.activation(out=pt[:, 0:cols],
                                     in_=s_big[:, 0:cols], func=Exp,
                                     scale=INV_SQRT_HS)
                for (kc, off, w, _pt, pcol) in pair:
                    if kc - 4 * qb >= 0:     # diagonal: one [128,128] tri mask
                        nc.vector.tensor_mul(pt[:, pcol:pcol + 128],
                                             pt[:, pcol:pcol + 128], tri_sb)
                # fold this pair into the group's fp32 denominator
                # accumulator (colsum(a) + colsum(b) == colsum(a + b));
                # diagonal blocks land query-aligned via shifted APs
                for (kc, off, w, _pt, pcol) in pair:
                    if lsum is None:
                        lsum = ls_pool.tile([128, 512], F32, tag="ls")
                        nc.vector.tensor_copy(out=lsum, in_=pt[:, pcol:pcol + w])
                    elif off == 0:
                        nc.vector.tensor_add(lsum, lsum, pt[:, pcol:pcol + w])
                    else:
                        nc.vector.tensor_add(lsum[:, off:512], lsum[:, off:512],
                                             pt[:, pcol:pcol + w])
                if prev is not None:
                    _emit_y(prev)
                prev = pair
            _emit_y(prev)
            ls16 = ls_pool.tile([128, 512], BF16, tag="ls16")
            nc.vector.tensor_copy(out=ls16, in_=lsum)
            nc.tensor.matmul(l_ps, ones_sb, ls16, start=True, stop=True,
                             skip_group_check=True)
            pending = (h, qb, y_ps, l_ps)
            # interleave one c_proj T-chunk of the PREVIOUS query block
            if qb > 0:
                _cproj_tm((qb - 1) * 4 + h)
    _attn_epilogue(*pending)
    for h in range(QH):
        _cproj_tm(12 + h, tail=(h >= 2))

    ph2.close()
    persist.close()


# ---------------------------------------------------------------- host side

def _rope_cache_np(seq_len, dim):
    inv_freq = 1.0 / (SCALE * BASE ** (np.arange(0, dim, 2, dtype=np.float32) / dim))
    t = np.arange(seq_len, dtype=np.float32)
    freqs = np.outer(t, inv_freq).astype(np.float32)
    emb = np.concatenate([freqs, freqs], axis=-1)
    return np.cos(emb).astype(np.float32), np.sin(emb).astype(np.float32)


_CACHE = {}


def _get_nc():
    if "nc" not in _CACHE:
        _CACHE["nc"] = _build_nc()
    return _CACHE["nc"]


def _f16(a):
    return np.ascontiguousarray(a.astype(np.float16))


def _bf(a):
    return np.ascontiguousarray(a.astype(ml_dtypes.bfloat16))


def kernel(q_x, Wq, bq, Wk, bk, Wv, bv, Wc, bc, _trace=False):
    q_x = np.asarray(q_x, dtype=np.float32)
    Wq = np.asarray(Wq, dtype=np.float32)
    Wk = np.asarray(Wk, dtype=np.float32)
    Wv = np.asarray(Wv, dtype=np.float32)
    Wc = np.asarray(Wc, dtype=np.float32)
    bq = np.asarray(bq, dtype=np.float32)
    bv = np.asarray(bv, dtype=np.float32)
    bc = np.asarray(bc, dtype=np.float32)
    # NOTE: bk is exactly softmax-invariant (adds a per-query constant to all
    # scores) so it is dropped on device.

    x = q_x.reshape(T, C)
    xt = _f16(x.T)                                       # [C, T] fp16

    cos, sin = _rope_cache_np(T, HS)                     # [T, 128]
    cosT = _f16(cos.T)                                   # [128, T]
    sinT = _f16(sin.T)

    ii = np.arange(128)
    tri = _bf((ii[:, None] <= ii[None, :]).astype(np.float32))
    ones = _bf(np.ones((128, 128), dtype=np.float32))
    ident = _bf(np.eye(128, dtype=np.float32))
    rmat = np.zeros((128, 128), dtype=np.float32)        # lhsT of rotate-half
    rmat[ii[:64] + 64, ii[:64]] = -1.0
    rmat[ii[:64], ii[:64] + 64] = 1.0
    rmat = _f16(rmat)

    in_maps = []
    for c in range(NCORES):
        wq_c = Wq[c * DQ:(c + 1) * DQ, :]                # [512, C]
        wk_c = Wk[c * HS:(c + 1) * HS, :]                # [128, C]
        wv_c = Wv[c * HS:(c + 1) * HS, :]
        wqkv = _f16(np.concatenate([wq_c, wk_c, wv_c], axis=0).T)  # [C, 768]
        wct_c = _f16(Wc[:, c * DQ:(c + 1) * DQ].T)       # [512, C]
        bcol = np.zeros((128, QH + 2), dtype=np.float32)
        bcol[:, :QH] = bq[c * DQ:(c + 1) * DQ].reshape(QH, 128).T
        bcol[:, QH] = bv[c * HS:(c + 1) * HS]
        bcol[:, QH + 1] = EXP_BIAS
        in_maps.append({
            "xt": xt, "wqkv": wqkv, "wct": wct_c, "cosT": cosT, "sinT": sinT,
            "tri": tri, "ones": ones, "ident": ident, "rmat": rmat,
            "bcol": bcol,
        })

    nc = _get_nc()
    res = run_bass_kernel_spmd(nc, in_maps, core_ids=list(range(NCORES)),
                               trace=_trace)
    acc = np.zeros((T, C), dtype=np.float64)
    for c in range(NCORES):
        acc += res.results[c]["out"].astype(np.float64)
    out = (acc + bc.astype(np.float64)).astype(np.float32)
    if _trace:
        _CACHE["last_exec_time_ns"] = res.exec_time_ns
        _CACHE["last_results"] = res
    return out.reshape(B, T, C)


# revision 34
# speedup vs baseline: 1.0101x; 1.0101x over previous
"""Trainium2 Bass kernel for a GQA attention block (B=1, T=2048, C=4096,
NH=32, NKV=8, HS=128), tensor-parallel over heads across 8 NeuronCores.

Per core c: 4 query heads (4c..4c+3) and 1 KV head (c). All matmul
operands are fp16 (full PE speed, FWL weight loads, half the HBM traffic
of fp32, 8x finer mantissa than bf16); PSUM accumulation stays fp32.

  - projections computed DIRECTLY in transposed layout: lhsT = W chunk
    (stationary), rhs = x^T chunk (moving, 512 wide)  ->  qT/kT/vT [HS, T]
  - c-chunk-major loop + engine-split DMA queues (x^T on GpSimd, weights
    on SP): first matmul only needs 1/32 of the weights, compute starts
    a few us in instead of waiting for the full weight DMA
  - RoPE on transposed tiles: rot = R @ qT via one PE matmul (R = exact
    +-1 rotate-half matrix), then qkT = qT*cosT + rot*sinT on DVE
  - V transposed back to natural [T, HS] via PE transpose (needed as the
    stationary operand of the P@V matmul)
  - attention on S^T blocks [keys, queries] with causally-reduced widths;
    exp batched in kc-pairs [128,1024] with bias -4 (keeps p in fp16
    range); softmax denominator via an all-ones matmul (broadcast over
    all 128 partitions for free); 1/l = exp(-ln l) on ScalarE (both in
    the natural_log_exp table set) instead of the slow DVE reciprocal
  - c_proj partial interleaved with the next query block's attention,
    drains split between DVE and ACT; partials summed on host (the TP
    all-reduce), fp16 on the wire.
"""
import sys

sys.path.insert(0, "/opt/trn_rl_repo")

import numpy as np
import ml_dtypes

from contextlib import ExitStack

import concourse.bass as bass
import concourse.mybir as mybir
import concourse.tile as tile
from concourse.bass_utils import run_bass_kernel_spmd

# ---------------------------------------------------------------- constants
B, T, C = 1, 2048, 4096
NH, NKV, HS = 32, 8, 128
NCORES = 8
QH = NH // NCORES          # 4 query heads per core
DQ = QH * HS               # 512
NKC = C // 128             # 32 contraction chunks
NTB = T // 512             # 4 T-blocks (= query blocks)
NTM = T // 128             # 16 T-chunks
BASE, SCALE = 10000.0, 1.0
INV_SQRT_HS = 1.0 / float(np.sqrt(HS))
EXP_BIAS = -4.0            # exp(s/sqrt(HS) - 4): scale-invariant in softmax,
                           # keeps p well inside fp16 range

F32 = mybir.dt.float32
F16 = mybir.dt.float16
BF16 = mybir.dt.bfloat16

# ------------------------------------------------------- wait legalization
_TAIL_RUNWAY = 48


def _legalize_waits(nc):
    """walrus (this toolchain) allows ONE sync wait per ISA instruction.
    Split excess waits off onto standalone EventSemaphore instructions
    inserted immediately before the offender (same engine stream order)."""
    n_split = 0
    for bb in nc.m.functions[0].blocks:
        insts = bb.instructions
        if not any(i.sync_info and i.sync_info.on_wait and
                   len(i.sync_info.on_wait) > (0 if type(i).__name__ == "InstISA" else 1)
                   for i in insts):
            continue
        new_list = []
        for inst in insts:
            si = inst.sync_info
            is_raw_isa = type(inst).__name__ == "InstISA"
            keep_n = 0 if is_raw_isa else 1
            if si and si.on_wait and len(si.on_wait) > keep_n:
                waits = list(si.on_wait)
                split_off = waits if is_raw_isa else waits[:-1]
                for w in split_off:
                    ev = mybir.InstNoOp(
                        name=f"legal-wait-{nc.next_id()}",
                        ins=[], outs=[], engine=inst.engine,
                        bass_nofuse=True,
                        sync_info=mybir.SyncInfo(on_wait=[w], on_update=[]))
                    nc.register_instruction(ev, overwrite=True)
                    new_list.append(ev)
                    n_split += 1
                inst.sync_info = mybir.SyncInfo(
                    on_wait=[] if is_raw_isa else [waits[-1]],
                    on_update=list(si.on_update))
            new_list.append(inst)
        bb.instructions = new_list
    return n_split


def _audit(nc):
    bad = []
    for bb in nc.m.functions[0].blocks:
        for inst in bb.instructions:
            si = inst.sync_info
            if si and si.on_wait and len(si.on_wait) > 1:
                bad.append((type(inst).__name__, inst.name, str(inst.engine),
                            len(si.on_wait)))
    return bad


class _TailRunwayPatch:
    """Plant runway nops on SP right before Tile's tail drain so the drain's
    many queue waits can be redistributed by _legalize_waits."""

    def __enter__(self):
        self.orig = tile.TileContext._drain_and_barrier
        orig = self.orig

        def patched(tc_self, tick_clock, wait_clock):
            for _ in range(_TAIL_RUNWAY):
                tc_self.nc.sync.nop(nofuse=True)
            return orig(tc_self, tick_clock, wait_clock)

        tile.TileContext._drain_and_barrier = patched
        return self

    def __exit__(self, *a):
        tile.TileContext._drain_and_barrier = self.orig


# ---------------------------------------------------------------- builder

def _build_nc():
    nc = bass.Bass(trn_type="TRN2")

    xt = nc.dram_tensor("xt", [C, T], F16, kind="ExternalInput")
    wqkv = nc.dram_tensor("wqkv", [C, DQ + 2 * HS], F16, kind="ExternalInput")
    wct = nc.dram_tensor("wct", [DQ, C], F16, kind="ExternalInput")
    cosT = nc.dram_tensor("cosT", [HS, T], F16, kind="ExternalInput")
    sinT = nc.dram_tensor("sinT", [HS, T], F16, kind="ExternalInput")
    tri = nc.dram_tensor("tri", [128, 128], BF16, kind="ExternalInput")
    ones = nc.dram_tensor("ones", [128, 128], BF16, kind="ExternalInput")
    ident = nc.dram_tensor("ident", [128, 128], BF16, kind="ExternalInput")
    rmat = nc.dram_tensor("rmat", [128, 128], F16, kind="ExternalInput")
    bcol = nc.dram_tensor("bcol", [128, QH + 2], F32, kind="ExternalInput")
    out = nc.dram_tensor("out", [T, C], F16, kind="ExternalOutput")

    with _TailRunwayPatch(), tile.TileContext(nc) as tc:
        _trace_body(nc, tc, xt, wqkv, wct, cosT, sinT, tri, ones, ident,
                    rmat, bcol, out)

    _legalize_waits(nc)
    bad = _audit(nc)
    if bad:
        raise RuntimeError(f"multi-wait instructions remain: {bad[:10]}")
    return nc


def _trace_body(nc, tc, xt, wqkv, wct, cosT, sinT, tri, ones, ident, rmat,
                bcol, out):
    Ident = mybir.ActivationFunctionType.Identity
    Exp = mybir.ActivationFunctionType.Exp
    Ln = mybir.ActivationFunctionType.Ln

    persist = ExitStack()
    misc = persist.enter_context(tc.tile_pool(name="misc", bufs=1))
    qkt_pool = persist.enter_context(tc.tile_pool(name="qkt", bufs=1))
    v_pool = persist.enter_context(tc.tile_pool(name="vsb", bufs=1))

    tri_sb = misc.tile([128, 128], BF16)
    ones_sb = misc.tile([128, 128], BF16)
    ident_sb = misc.tile([128, 128], BF16)
    rmat_sb = misc.tile([128, 128], F16)
    bcol_sb = misc.tile([128, QH + 2], F32)

    qT = qkt_pool.tile([128, QH, T], F16)      # per-head q^T [HS, T], RoPE'd
    kT = qkt_pool.tile([128, T], F16)          # k^T [HS, T], RoPE'd
    v_sb = v_pool.tile([128, NTM, HS], BF16)    # V natural [T, HS]

    # ================= phase 1: projections + RoPE (transposed) ==========
    ph1 = ExitStack()
    w_pool = ph1.enter_context(tc.tile_pool(name="wqkv", bufs=1))
    xt_pool = ph1.enter_context(tc.tile_pool(name="xt", bufs=3))
    cs_pool = ph1.enter_context(tc.tile_pool(name="cossin", bufs=1))
    sc_pool = ph1.enter_context(tc.tile_pool(name="ropescr", bufs=4))
    tmp_pool = ph1.enter_context(tc.tile_pool(name="ropetmp", bufs=4))
    vt_pool = ph1.enter_context(tc.tile_pool(name="vtscr", bufs=3))
    ps_pr = ph1.enter_context(tc.tile_pool(name="pspr", bufs=6, space="PSUM"))
    ps_rot = ph1.enter_context(tc.tile_pool(name="psrot", bufs=2, space="PSUM"))

    w0_pool = ph1.enter_context(tc.tile_pool(name="w0", bufs=1))
    wqkv_sb = w_pool.tile([128, NKC, DQ + 2 * HS], F16)
    # x^T tiles ride the (otherwise idle) GpSimd queue; weights ride SP.
    # Interleave the first T-block's x chunks with the weight chunks so the
    # projection's c-chunk-major loop can start after ~2 chunks have landed.
    # Chunk 0 gets its own tiny tiles so the very first matmul has a precise
    # two-DMA dependency instead of a coarse multi-chunk semaphore threshold.
    w0_sb = w0_pool.tile([128, DQ + 2 * HS], F16)
    nc.sync.dma_start(out=w0_sb, in_=wqkv[0:128, :])
    x0_sb = w0_pool.tile([128, 512], F16)
    nc.gpsimd.dma_start(out=x0_sb, in_=xt[0:128, 0:512])
    xt_tiles = [None] * NTB
    xt_tiles[0] = xt_pool.tile([128, NKC, 512], F16, tag="xt", name="xt_sb")
    for kc in range(1, NKC):
        nc.sync.dma_start(out=wqkv_sb[:, kc, :],
                          in_=wqkv[kc * 128:(kc + 1) * 128, :])
        nc.gpsimd.dma_start(out=xt_tiles[0][:, kc, :],
                            in_=xt[kc * 128:(kc + 1) * 128, 0:512])
    nc.sync.dma_start(out=wqkv_sb[:, 0, :], in_=wqkv[0:128, :])
    cosT_sb = cs_pool.tile([128, T], F16)
    nc.scalar.dma_start(out=cosT_sb, in_=cosT[:, :])
    sinT_sb = cs_pool.tile([128, T], F16)
    nc.scalar.dma_start(out=sinT_sb, in_=sinT[:, :])
    nc.scalar.dma_start(out=rmat_sb, in_=rmat[:, :])
    nc.scalar.dma_start(out=bcol_sb, in_=bcol[:, :])
    nc.scalar.dma_start(out=tri_sb, in_=tri[:, :])
    nc.scalar.dma_start(out=ones_sb, in_=ones[:, :])
    nc.scalar.dma_start(out=ident_sb, in_=ident[:, :])

    NOC = QH + 2  # out chunks: q0..q3, k, v

    for tb in range(NTB):
        t0 = tb * 512
        if xt_tiles[tb] is None:
            xt_tiles[tb] = xt_pool.tile([128, NKC, 512], F16, tag="xt", name="xt_sb")
            for kc in range(NKC):
                nc.gpsimd.dma_start(out=xt_tiles[tb][:, kc, :],
                                    in_=xt[kc * 128:(kc + 1) * 128,
                                           t0:t0 + 512])
        xt_sb = xt_tiles[tb]
        # 6 accumulators [128, 512], one psum bank each
        ps = []
        for _oc in range(NOC):
            pr_acc = ps_pr.tile([128, 512], F32, tag="pr")
            ps.append(pr_acc)
        for kc in range(NKC):
            first = (tb == 0 and kc == 0)
            wsrc = w0_sb if first else wqkv_sb[:, kc, :]
            xsrc = x0_sb if first else xt_sb[:, kc, :]
            for oc in range(NOC):
                nc.tensor.matmul(ps[oc], wsrc[:, oc * 128:(oc + 1) * 128]
                                 if first else
                                 wqkv_sb[:, kc, oc * 128:(oc + 1) * 128],
                                 xsrc,
                                 start=(kc == 0), stop=(kc == NKC - 1),
                                 skip_group_check=True)

        # ---- RoPE for q0..q3 and k (oc = 0..QH) ----
        for oc in range(QH + 1):
            src = sc_pool.tile([128, 512], F16, tag="scr")
            if oc < QH:
                nc.scalar.activation(out=src, in_=ps[oc], func=Ident,
                                     bias=bcol_sb[:, oc:oc + 1], scale=1.0)
            else:
                nc.scalar.copy(out=src, in_=ps[oc])  # k: bias dropped (softmax-invariant)
            rot = ps_rot.tile([128, 512], F32, tag="rot")
            nc.tensor.matmul(rot, rmat_sb, src, start=True, stop=True,
                             skip_group_check=True)
            a = tmp_pool.tile([128, 512], F16, tag="tmp")
            nc.vector.tensor_mul(a, src, cosT_sb[:, t0:t0 + 512])
            b = tmp_pool.tile([128, 512], F16, tag="tmp")
            nc.vector.tensor_mul(b, rot, sinT_sb[:, t0:t0 + 512])
            dst = qT[:, oc, t0:t0 + 512] if oc < QH else kT[:, t0:t0 + 512]
            nc.vector.tensor_add(dst, a, b)

        # ---- V: drain (bias) then transpose to natural [T, HS] ----
        vt_sb = vt_pool.tile([128, 512], BF16)
        nc.scalar.activation(out=vt_sb, in_=ps[QH + 1], func=Ident,
                             bias=bcol_sb[:, QH:QH + 1], scale=1.0)
        vtr = ps_rot.tile([128, 512], BF16, tag="rot")
        for j in range(4):
            nc.tensor.matmul(vtr[:, j * 128:(j + 1) * 128],
                             vt_sb[:, j * 128:(j + 1) * 128], ident_sb,
                             is_transpose=True, skip_group_check=True)
        nc.vector.tensor_copy(out=v_sb[:, 4 * tb:4 * tb + 4, :], in_=vtr)

    ph1.close()

    # ============== phase 2: attention + c_proj, interleaved =============
    ph2 = ExitStack()
    wc_pool = ph2.enter_context(tc.tile_pool(name="wct", bufs=1))
    yt_pool = ph2.enter_context(tc.tile_pool(name="yt", bufs=1))
    pt_pool = ph2.enter_context(tc.tile_pool(name="pt", bufs=6))
    ls_pool = ph2.enter_context(tc.tile_pool(name="lsum", bufs=3))
    lw_pool = ph2.enter_context(tc.tile_pool(name="lwork", bufs=3))
    out_pool = ph2.enter_context(tc.tile_pool(name="outsb", bufs=8))
    ps_s = ph2.enter_context(tc.tile_pool(name="pss", bufs=2, space="PSUM"))
    ps_y = ph2.enter_context(tc.tile_pool(name="psy", bufs=1, space="PSUM"))
    ps_l = ph2.enter_context(tc.tile_pool(name="psl", bufs=1, space="PSUM"))
    ps_o = ph2.enter_context(tc.tile_pool(name="pso", bufs=2, space="PSUM"))

    wct_sb = wc_pool.tile([128, QH, C], F16)
    for h in range(QH):
        nc.sync.dma_start(out=wct_sb[:, h, :],
                          in_=wct[h * 128:(h + 1) * 128, :])
    yT = yt_pool.tile([128, QH, T], F16)

    def _attn_epilogue(h, qb, y_ps, l_ps):
        # 1/l = exp(-ln l) on ACT (exp & ln share one table set); the slow
        # DVE reciprocal serialized the whole pipeline.
        lnl = lw_pool.tile([128, 512], F32, tag="lw")
        nc.scalar.activation(out=lnl, in_=l_ps, func=Ln, scale=1.0)
        linv = lw_pool.tile([128, 512], F32, tag="lw")
        nc.scalar.activation(out=linv, in_=lnl, func=Exp, scale=-1.0)
        nc.vector.tensor_mul(yT[:, h, qb * 512:(qb + 1) * 512], y_ps, linv)

    def _cproj_tm(tm, tail=False):
        for oc in range(8):
            o_ps = ps_o.tile([128, 512], F32)
            for h in range(QH):
                nc.tensor.matmul(o_ps, yT[:, h, tm * 128:(tm + 1) * 128],
                                 wct_sb[:, h, oc * 512:(oc + 1) * 512],
                                 start=(h == 0), stop=(h == QH - 1),
                                 skip_group_check=True)
            out_sb = out_pool.tile([128, 512], F16)
            if (oc % 2 == 0) if tail else (oc < 6):
                nc.vector.tensor_copy(out=out_sb, in_=o_ps)
            else:
                nc.scalar.copy(out=out_sb, in_=o_ps)
            dst = out[tm * 128:(tm + 1) * 128, oc * 512:(oc + 1) * 512]
            if tail and oc % 2 == 1:
                nc.sync.dma_start(out=dst, in_=out_sb)
            else:
                nc.gpsimd.dma_start(out=dst, in_=out_sb)

    pending = None
    for qb in range(NTB):
        for h in range(QH):
            if pending is not None:
                _attn_epilogue(*pending)
                pending = None
            nkc = 4 * (qb + 1)
            y_ps = ps_y.tile([128, 512], F32)
            l_ps = ps_l.tile([128, 512], F32)
            prev = None  # previous kc-pair, its y/l matmuls emitted after
                         # the next pair's exp (software skew)

            l_started = [False]

            def _emit_yl(pair, lsum):
                for (kc, off, w, pt, pcol) in pair:
                    nc.tensor.matmul(y_ps[:, off:512], v_sb[:, kc, :],
                                     pt[:, pcol:pcol + w], start=(kc == 0),
                                     stop=(kc == nkc - 1),
                                     skip_group_check=True)
                if lsum is False:
                    pass                     # denominator folded downstream
                elif lsum is not None:
                    # folded full pairs: one denominator matmul on the
                    # running sum (colsum(a) + colsum(b) == colsum(a + b))
                    nc.tensor.matmul(l_ps, ones_sb, lsum,
                                     start=(not l_started[0]), stop=False,
                                     skip_group_check=True)
                    l_started[0] = True
                else:
                    for (kc, off, w, pt, pcol) in pair:
                        nc.tensor.matmul(l_ps[:, off:512], ones_sb,
                                         pt[:, pcol:pcol + w],
                                         start=(not l_started[0]),
                                         stop=(kc == nkc - 1),
                                         skip_group_check=True)
                        l_started[0] = True

            for p0 in range(0, nkc, 2):
                s_big = ps_s.tile([128, 1024], F32, tag="s")
                pt = pt_pool.tile([128, 1024], BF16, tag="pt")
                pair = []
                cols = 0
                for kc in (p0, p0 + 1):
                    o = kc - 4 * qb          # >=0 on diagonal blocks
                    off = max(o, 0) * 128    # first live query column
                    w = 512 - off
                    nc.tensor.matmul(s_big[:, cols:cols + w],
                                     kT[:, kc * 128:(kc + 1) * 128],
                                     qT[:, h, qb * 512 + off:(qb + 1) * 512],
                                     start=True, stop=True,
                                     skip_group_check=True)
                    pair.append((kc, off, w, pt, cols))
                    cols += w
                # one exp over both packed blocks
                nc.scalar.activation(out=pt[:, 0:cols],
                                     in_=s_big[:, 0:cols], func=Exp,
                                     scale=INV_SQRT_HS)
                for (kc, off, w, _pt, pcol) in pair:
                    if kc - 4 * qb >= 0:     # diagonal: one [128,128] tri mask
                        nc.vector.tensor_mul(pt[:, pcol:pcol + 128],
                                             pt[:, pcol:pcol + 128], tri_sb)
                lsum = None
                if p0 + 1 < 4 * qb:          # both blocks full width
                    lsum = ls_pool.tile([128, 512], BF16, tag="ls")
                    nc.vector.tensor_add(lsum, pt[:, 0:512], pt[:, 512:1024])
                    if prev is not None and prev[1] is not None:
                        # quad-fold: merge with the previous full pair's sum
                        # and emit its y-matmuls with no denominator matmul
                        nc.vector.tensor_add(lsum, lsum, prev[1])
                        _emit_yl(prev[0], False)
                        prev = None
                if prev is not None:
                    _emit_yl(*prev)
                prev = (pair, lsum)
            _emit_yl(*prev)
            pending = (h, qb, y_ps, l_ps)
            # interleave one c_proj T-chunk of the PREVIOUS query block
            if qb > 0:
                _cproj_tm((qb - 1) * 4 + h)
    _attn_epilogue(*pending)
    for h in range(QH):
        _cproj_tm(12 + h, tail=(h >= 2))

    ph2.close()
    persist.close()


# ---------------------------------------------------------------- host side

def _rope_cache_np(seq_len, dim):
    inv_freq = 1.0 / (SCALE * BASE ** (np.arange(0, dim, 2, dtype=np.float32) / dim))
    t = np.arange(seq_len, dtype=np.float32)
    freqs = np.outer(t, inv_freq).astype(np.float32)
    emb = np.concatenate([freqs, freqs], axis=-1)
    return np.cos(emb).astype(np.float32), np.sin(emb).astype(np.float32)


_CACHE = {}


def _get_nc():
    if "nc" not in _CACHE:
        _CACHE["nc"] = _build_nc()
    return _CACHE["nc"]


def _f16(a):
    return np.ascontiguousarray(a.astype(np.float16))


def _bf(a):
    return np.ascontiguousarray(a.astype(ml_dtypes.bfloat16))


def kernel(q_x, Wq, bq, Wk, bk, Wv, bv, Wc, bc, _trace=False):
    q_x = np.asarray(q_x, dtype=np.float32)
    Wq = np.asarray(Wq, dtype=np.float32)
    Wk = np.asarray(Wk, dtype=np.float32)
    Wv = np.asarray(Wv, dtype=np.float32)
    Wc = np.asarray(Wc, dtype=np.float32)
    bq = np.asarray(bq, dtype=np.float32)
    bv = np.asarray(bv, dtype=np.float32)
    bc = np.asarray(bc, dtype=np.float32)
    # NOTE: bk is exactly softmax-invariant (adds a per-query constant to all
    # scores) so it is dropped on device.

    x = q_x.reshape(T, C)
    xt = _f16(x.T)                                       # [C, T] fp16

    cos, sin = _rope_cache_np(T, HS)                     # [T, 128]
    cosT = _f16(cos.T)                                   # [128, T]
    sinT = _f16(sin.T)

    ii = np.arange(128)
    tri = _bf((ii[:, None] <= ii[None, :]).astype(np.float32))
    ones = _bf(np.ones((128, 128), dtype=np.float32))
    ident = _bf(np.eye(128, dtype=np.float32))
    rmat = np.zeros((128, 128), dtype=np.float32)        # lhsT of rotate-half
    rmat[ii[:64] + 64, ii[:64]] = -1.0
    rmat[ii[:64], ii[:64] + 64] = 1.0
    rmat = _f16(rmat)

    in_maps = []
    for c in range(NCORES):
        wq_c = Wq[c * DQ:(c + 1) * DQ, :]                # [512, C]
        wk_c = Wk[c * HS:(c + 1) * HS, :]                # [128, C]
        wv_c = Wv[c * HS:(c + 1) * HS, :]
        wqkv = _f16(np.concatenate([wq_c, wk_c, wv_c], axis=0).T)  # [C, 768]
        wct_c = _f16(Wc[:, c * DQ:(c + 1) * DQ].T)       # [512, C]
        bcol = np.zeros((128, QH + 2), dtype=np.float32)
        bcol[:, :QH] = bq[c * DQ:(c + 1) * DQ].reshape(QH, 128).T
        bcol[:, QH] = bv[c * HS:(c + 1) * HS]
        bcol[:, QH + 1] = EXP_BIAS
        in_maps.append({
            "xt": xt, "wqkv": wqkv, "wct": wct_c, "cosT": cosT, "sinT": sinT,
            "tri": tri, "ones": ones, "ident": ident, "rmat": rmat,
            "bcol": bcol,
        })

    nc = _get_nc()
    res = run_bass_kernel_spmd(nc, in_maps, core_ids=list(range(NCORES)),
                               trace=_trace)
    acc = np.zeros((T, C), dtype=np.float64)
    for c in range(NCORES):
        acc += res.results[c]["out"].astype(np.float64)
    out = (acc + bc.astype(np.float64)).astype(np.float32)
    if _trace:
        _CACHE["last_exec_time_ns"] = res.exec_time_ns
        _CACHE["last_results"] = res
    return out.reshape(B, T, C)


# revision 35
# speedup vs baseline: 1.0167x; 1.0065x over previous
"""Trainium2 Bass kernel for a GQA attention block (B=1, T=2048, C=4096,
NH=32, NKV=8, HS=128), tensor-parallel over heads across 8 NeuronCores.

Per core c: 4 query heads (4c..4c+3) and 1 KV head (c). All matmul
operands are fp16 (full PE speed, FWL weight loads, half the HBM traffic
of fp32, 8x finer mantissa than bf16); PSUM accumulation stays fp32.

  - projections computed DIRECTLY in transposed layout: lhsT = W chunk
    (stationary), rhs = x^T chunk (moving, 512 wide)  ->  qT/kT/vT [HS, T]
  - c-chunk-major loop + engine-split DMA queues (x^T on GpSimd, weights
    on SP): first matmul only needs 1/32 of the weights, compute starts
    a few us in instead of waiting for the full weight DMA
  - RoPE on transposed tiles: rot = R @ qT via one PE matmul (R = exact
    +-1 rotate-half matrix), then qkT = qT*cosT + rot*sinT on DVE
  - V transposed back to natural [T, HS] via PE transpose (needed as the
    stationary operand of the P@V matmul)
  - attention on S^T blocks [keys, queries] with causally-reduced widths;
    exp batched in kc-pairs [128,1024] with bias -4 (keeps p in fp16
    range); softmax denominator via an all-ones matmul (broadcast over
    all 128 partitions for free); 1/l = exp(-ln l) on ScalarE (both in
    the natural_log_exp table set) instead of the slow DVE reciprocal
  - c_proj partial interleaved with the next query block's attention,
    drains split between DVE and ACT; partials summed on host (the TP
    all-reduce), fp16 on the wire.
"""
import sys

sys.path.insert(0, "/opt/trn_rl_repo")

import numpy as np
import ml_dtypes

from contextlib import ExitStack

import concourse.bass as bass
import concourse.mybir as mybir
import concourse.tile as tile
from concourse.bass_utils import run_bass_kernel_spmd

# ---------------------------------------------------------------- constants
B, T, C = 1, 2048, 4096
NH, NKV, HS = 32, 8, 128
NCORES = 8
QH = NH // NCORES          # 4 query heads per core
DQ = QH * HS               # 512
NKC = C // 128             # 32 contraction chunks
NTB = T // 512             # 4 T-blocks (= query blocks)
NTM = T // 128             # 16 T-chunks
BASE, SCALE = 10000.0, 1.0
INV_SQRT_HS = 1.0 / float(np.sqrt(HS))
EXP_BIAS = -4.0            # exp(s/sqrt(HS) - 4): scale-invariant in softmax,
                           # keeps p well inside fp16 range

F32 = mybir.dt.float32
F16 = mybir.dt.float16
BF16 = mybir.dt.bfloat16

# ------------------------------------------------------- wait legalization
_TAIL_RUNWAY = 48


def _legalize_waits(nc):
    """walrus (this toolchain) allows ONE sync wait per ISA instruction.
    Split excess waits off onto standalone EventSemaphore instructions
    inserted immediately before the offender (same engine stream order)."""
    n_split = 0
    for bb in nc.m.functions[0].blocks:
        insts = bb.instructions
        if not any(i.sync_info and i.sync_info.on_wait and
                   len(i.sync_info.on_wait) > (0 if type(i).__name__ == "InstISA" else 1)
                   for i in insts):
            continue
        new_list = []
        for inst in insts:
            si = inst.sync_info
            is_raw_isa = type(inst).__name__ == "InstISA"
            keep_n = 0 if is_raw_isa else 1
            if si and si.on_wait and len(si.on_wait) > keep_n:
                waits = list(si.on_wait)
                split_off = waits if is_raw_isa else waits[:-1]
                for w in split_off:
                    ev = mybir.InstNoOp(
                        name=f"legal-wait-{nc.next_id()}",
                        ins=[], outs=[], engine=inst.engine,
                        bass_nofuse=True,
                        sync_info=mybir.SyncInfo(on_wait=[w], on_update=[]))
                    nc.register_instruction(ev, overwrite=True)
                    new_list.append(ev)
                    n_split += 1
                inst.sync_info = mybir.SyncInfo(
                    on_wait=[] if is_raw_isa else [waits[-1]],
                    on_update=list(si.on_update))
            new_list.append(inst)
        bb.instructions = new_list
    return n_split


def _audit(nc):
    bad = []
    for bb in nc.m.functions[0].blocks:
        for inst in bb.instructions:
            si = inst.sync_info
            if si and si.on_wait and len(si.on_wait) > 1:
                bad.append((type(inst).__name__, inst.name, str(inst.engine),
                            len(si.on_wait)))
    return bad


class _TailRunwayPatch:
    """Plant runway nops on SP right before Tile's tail drain so the drain's
    many queue waits can be redistributed by _legalize_waits."""

    def __enter__(self):
        self.orig = tile.TileContext._drain_and_barrier
        orig = self.orig

        def patched(tc_self, tick_clock, wait_clock):
            for _ in range(_TAIL_RUNWAY):
                tc_self.nc.sync.nop(nofuse=True)
            return orig(tc_self, tick_clock, wait_clock)

        tile.TileContext._drain_and_barrier = patched
        return self

    def __exit__(self, *a):
        tile.TileContext._drain_and_barrier = self.orig


# ---------------------------------------------------------------- builder

def _build_nc():
    nc = bass.Bass(trn_type="TRN2")

    xt = nc.dram_tensor("xt", [C, T], F16, kind="ExternalInput")
    wqkv = nc.dram_tensor("wqkv", [C, DQ + 2 * HS], F16, kind="ExternalInput")
    wct = nc.dram_tensor("wct", [DQ, C], F16, kind="ExternalInput")
    cosT = nc.dram_tensor("cosT", [HS, T], F16, kind="ExternalInput")
    sinT = nc.dram_tensor("sinT", [HS, T], F16, kind="ExternalInput")
    tri = nc.dram_tensor("tri", [128, 128], BF16, kind="ExternalInput")
    ones = nc.dram_tensor("ones", [128, 128], BF16, kind="ExternalInput")
    ident = nc.dram_tensor("ident", [128, 128], BF16, kind="ExternalInput")
    rmat = nc.dram_tensor("rmat", [128, 128], F16, kind="ExternalInput")
    bcol = nc.dram_tensor("bcol", [128, QH + 2], F32, kind="ExternalInput")
    out = nc.dram_tensor("out", [T, C], F16, kind="ExternalOutput")

    with _TailRunwayPatch(), tile.TileContext(nc) as tc:
        _trace_body(nc, tc, xt, wqkv, wct, cosT, sinT, tri, ones, ident,
                    rmat, bcol, out)

    _legalize_waits(nc)
    bad = _audit(nc)
    if bad:
        raise RuntimeError(f"multi-wait instructions remain: {bad[:10]}")
    return nc


def _trace_body(nc, tc, xt, wqkv, wct, cosT, sinT, tri, ones, ident, rmat,
                bcol, out):
    Ident = mybir.ActivationFunctionType.Identity
    Exp = mybir.ActivationFunctionType.Exp
    Ln = mybir.ActivationFunctionType.Ln

    persist = ExitStack()
    misc = persist.enter_context(tc.tile_pool(name="misc", bufs=1))
    qkt_pool = persist.enter_context(tc.tile_pool(name="qkt", bufs=1))
    v_pool = persist.enter_context(tc.tile_pool(name="vsb", bufs=1))

    tri_sb = misc.tile([128, 128], BF16)
    ones_sb = misc.tile([128, 128], BF16)
    ident_sb = misc.tile([128, 128], BF16)
    rmat_sb = misc.tile([128, 128], F16)
    bcol_sb = misc.tile([128, QH + 2], F32)

    qT = qkt_pool.tile([128, QH, T], F16)      # per-head q^T [HS, T], RoPE'd
    kT = qkt_pool.tile([128, T], F16)          # k^T [HS, T], RoPE'd
    v_sb = v_pool.tile([128, NTM, HS], BF16)    # V natural [T, HS]

    # ================= phase 1: projections + RoPE (transposed) ==========
    ph1 = ExitStack()
    w_pool = ph1.enter_context(tc.tile_pool(name="wqkv", bufs=1))
    xt_pool = ph1.enter_context(tc.tile_pool(name="xt", bufs=3))
    cs_pool = ph1.enter_context(tc.tile_pool(name="cossin", bufs=1))
    sc_pool = ph1.enter_context(tc.tile_pool(name="ropescr", bufs=4))
    tmp_pool = ph1.enter_context(tc.tile_pool(name="ropetmp", bufs=4))
    vt_pool = ph1.enter_context(tc.tile_pool(name="vtscr", bufs=3))
    ps_pr = ph1.enter_context(tc.tile_pool(name="pspr", bufs=6, space="PSUM"))
    ps_rot = ph1.enter_context(tc.tile_pool(name="psrot", bufs=2, space="PSUM"))

    w0_pool = ph1.enter_context(tc.tile_pool(name="w0", bufs=1))
    wqkv_sb = w_pool.tile([128, NKC, DQ + 2 * HS], F16)
    # x^T tiles ride the (otherwise idle) GpSimd queue; weights ride SP.
    # Interleave the first T-block's x chunks with the weight chunks so the
    # projection's c-chunk-major loop can start after ~2 chunks have landed.
    # Chunk 0 gets its own tiny tiles so the very first matmul has a precise
    # two-DMA dependency instead of a coarse multi-chunk semaphore threshold.
    w0_sb = w0_pool.tile([128, DQ + 2 * HS], F16)
    nc.sync.dma_start(out=w0_sb, in_=wqkv[0:128, :])
    x0_sb = w0_pool.tile([128, 512], F16)
    nc.gpsimd.dma_start(out=x0_sb, in_=xt[0:128, 0:512])
    xt_tiles = [None] * NTB
    xt_tiles[0] = xt_pool.tile([128, NKC, 512], F16, tag="xt", name="xt_sb")
    for kc in range(1, NKC):
        nc.sync.dma_start(out=wqkv_sb[:, kc, :],
                          in_=wqkv[kc * 128:(kc + 1) * 128, :])
        nc.gpsimd.dma_start(out=xt_tiles[0][:, kc, :],
                            in_=xt[kc * 128:(kc + 1) * 128, 0:512])
    nc.sync.dma_start(out=wqkv_sb[:, 0, :], in_=wqkv[0:128, :])
    cosT_sb = cs_pool.tile([128, T], F16)
    nc.scalar.dma_start(out=cosT_sb, in_=cosT[:, :])
    sinT_sb = cs_pool.tile([128, T], F16)
    nc.scalar.dma_start(out=sinT_sb, in_=sinT[:, :])
    nc.scalar.dma_start(out=rmat_sb, in_=rmat[:, :])
    nc.scalar.dma_start(out=bcol_sb, in_=bcol[:, :])
    nc.scalar.dma_start(out=tri_sb, in_=tri[:, :])
    nc.scalar.dma_start(out=ones_sb, in_=ones[:, :])
    nc.scalar.dma_start(out=ident_sb, in_=ident[:, :])

    NOC = QH + 2  # out chunks: q0..q3, k, v

    for tb in range(NTB):
        t0 = tb * 512
        if xt_tiles[tb] is None:
            xt_tiles[tb] = xt_pool.tile([128, NKC, 512], F16, tag="xt", name="xt_sb")
            for kc in range(NKC):
                nc.gpsimd.dma_start(out=xt_tiles[tb][:, kc, :],
                                    in_=xt[kc * 128:(kc + 1) * 128,
                                           t0:t0 + 512])
        xt_sb = xt_tiles[tb]
        # 6 accumulators [128, 512], one psum bank each
        ps = []
        for _oc in range(NOC):
            pr_acc = ps_pr.tile([128, 512], F32, tag="pr")
            ps.append(pr_acc)
        for kc in range(NKC):
            first = (tb == 0 and kc == 0)
            wsrc = w0_sb if first else wqkv_sb[:, kc, :]
            xsrc = x0_sb if first else xt_sb[:, kc, :]
            for oc in range(NOC):
                nc.tensor.matmul(ps[oc], wsrc[:, oc * 128:(oc + 1) * 128]
                                 if first else
                                 wqkv_sb[:, kc, oc * 128:(oc + 1) * 128],
                                 xsrc,
                                 start=(kc == 0), stop=(kc == NKC - 1),
                                 skip_group_check=True)

        # ---- RoPE for q0..q3 and k (oc = 0..QH) ----
        for oc in range(QH + 1):
            src = sc_pool.tile([128, 512], F16, tag="scr")
            if oc < QH:
                nc.scalar.activation(out=src, in_=ps[oc], func=Ident,
                                     bias=bcol_sb[:, oc:oc + 1], scale=1.0)
            else:
                nc.scalar.copy(out=src, in_=ps[oc])  # k: bias dropped (softmax-invariant)
            rot = ps_rot.tile([128, 512], F32, tag="rot")
            nc.tensor.matmul(rot, rmat_sb, src, start=True, stop=True,
                             skip_group_check=True)
            a = tmp_pool.tile([128, 512], F16, tag="tmp")
            nc.vector.tensor_mul(a, src, cosT_sb[:, t0:t0 + 512])
            b = tmp_pool.tile([128, 512], F16, tag="tmp")
            nc.vector.tensor_mul(b, rot, sinT_sb[:, t0:t0 + 512])
            dst = qT[:, oc, t0:t0 + 512] if oc < QH else kT[:, t0:t0 + 512]
            nc.vector.tensor_add(dst, a, b)

        # ---- V: drain (bias) then transpose to natural [T, HS] ----
        vt_sb = vt_pool.tile([128, 512], BF16)
        nc.scalar.activation(out=vt_sb, in_=ps[QH + 1], func=Ident,
                             bias=bcol_sb[:, QH:QH + 1], scale=1.0)
        vtr = ps_rot.tile([128, 512], BF16, tag="rot")
        for j in range(4):
            nc.tensor.matmul(vtr[:, j * 128:(j + 1) * 128],
                             vt_sb[:, j * 128:(j + 1) * 128], ident_sb,
                             is_transpose=True, skip_group_check=True)
        nc.vector.tensor_copy(out=v_sb[:, 4 * tb:4 * tb + 4, :], in_=vtr)

    ph1.close()

    # ============== phase 2: attention + c_proj, interleaved =============
    ph2 = ExitStack()
    wc_pool = ph2.enter_context(tc.tile_pool(name="wct", bufs=1))
    yt_pool = ph2.enter_context(tc.tile_pool(name="yt", bufs=1))
    pt_pool = ph2.enter_context(tc.tile_pool(name="pt", bufs=6))
    ls_pool = ph2.enter_context(tc.tile_pool(name="lsum", bufs=3))
    lw_pool = ph2.enter_context(tc.tile_pool(name="lwork", bufs=3))
    out_pool = ph2.enter_context(tc.tile_pool(name="outsb", bufs=8))
    ps_s = ph2.enter_context(tc.tile_pool(name="pss", bufs=2, space="PSUM"))
    ps_y = ph2.enter_context(tc.tile_pool(name="psy", bufs=1, space="PSUM"))
    ps_l = ph2.enter_context(tc.tile_pool(name="psl", bufs=1, space="PSUM"))
    ps_o = ph2.enter_context(tc.tile_pool(name="pso", bufs=2, space="PSUM"))

    wct_sb = wc_pool.tile([128, QH, C], F16)
    for h in range(QH):
        nc.sync.dma_start(out=wct_sb[:, h, :],
                          in_=wct[h * 128:(h + 1) * 128, :])
    yT = yt_pool.tile([128, QH, T], F16)

    def _attn_epilogue(h, qb, y_ps, l_ps):
        # 1/l = exp(-ln l) on ACT (exp & ln share one table set); the slow
        # DVE reciprocal serialized the whole pipeline.
        lnl = lw_pool.tile([128, 512], F32, tag="lw")
        nc.scalar.activation(out=lnl, in_=l_ps, func=Ln, scale=1.0)
        linv = lw_pool.tile([128, 512], F32, tag="lw")
        nc.scalar.activation(out=linv, in_=lnl, func=Exp, scale=-1.0)
        nc.vector.tensor_mul(yT[:, h, qb * 512:(qb + 1) * 512], y_ps, linv)

    def _cproj_tm(tm, tail=False):
        for oc in range(8):
            o_ps = ps_o.tile([128, 512], F32)
            for h in range(QH):
                nc.tensor.matmul(o_ps, yT[:, h, tm * 128:(tm + 1) * 128],
                                 wct_sb[:, h, oc * 512:(oc + 1) * 512],
                                 start=(h == 0), stop=(h == QH - 1),
                                 skip_group_check=True)
            out_sb = out_pool.tile([128, 512], F16)
            if (oc % 2 == 0) if tail else (oc < 6):
                nc.vector.tensor_copy(out=out_sb, in_=o_ps)
            else:
                nc.scalar.copy(out=out_sb, in_=o_ps)
            dst = out[tm * 128:(tm + 1) * 128, oc * 512:(oc + 1) * 512]
            if tail and oc % 2 == 1:
                nc.sync.dma_start(out=dst, in_=out_sb)
            else:
                nc.gpsimd.dma_start(out=dst, in_=out_sb)

    pending = None
    for qb in range(NTB):
        for h in range(QH):
            if pending is not None:
                _attn_epilogue(*pending)
                pending = None
            nkc = 4 * (qb + 1)
            y_ps = ps_y.tile([128, 512], F32)
            l_ps = ps_l.tile([128, 512], F32)
            diag_elems = []
            prev = None  # previous kc-pair, its y/l matmuls emitted after
                         # the next pair's exp (software skew)

            l_started = [False]

            def _emit_yl(pair, lsum):
                for (kc, off, w, pt, pcol) in pair:
                    nc.tensor.matmul(y_ps[:, off:512], v_sb[:, kc, :],
                                     pt[:, pcol:pcol + w], start=(kc == 0),
                                     stop=(kc == nkc - 1),
                                     skip_group_check=True)
                if lsum is False or lsum == "DF":
                    pass                     # denominator folded downstream
                elif lsum is not None:
                    # folded full pairs: one denominator matmul on the
                    # running sum (colsum(a) + colsum(b) == colsum(a + b))
                    nc.tensor.matmul(l_ps, ones_sb, lsum,
                                     start=(not l_started[0]), stop=False,
                                     skip_group_check=True)
                    l_started[0] = True
                else:
                    for (kc, off, w, pt, pcol) in pair:
                        nc.tensor.matmul(l_ps[:, off:512], ones_sb,
                                         pt[:, pcol:pcol + w],
                                         start=(not l_started[0]),
                                         stop=(kc == nkc - 1),
                                         skip_group_check=True)
                        l_started[0] = True

            for p0 in range(0, nkc, 2):
                s_big = ps_s.tile([128, 1024], F32, tag="s")
                pt = pt_pool.tile([128, 1024], BF16, tag="pt")
                pair = []
                cols = 0
                for kc in (p0, p0 + 1):
                    o = kc - 4 * qb          # >=0 on diagonal blocks
                    off = max(o, 0) * 128    # first live query column
                    w = 512 - off
                    nc.tensor.matmul(s_big[:, cols:cols + w],
                                     kT[:, kc * 128:(kc + 1) * 128],
                                     qT[:, h, qb * 512 + off:(qb + 1) * 512],
                                     start=True, stop=True,
                                     skip_group_check=True)
                    pair.append((kc, off, w, pt, cols))
                    cols += w
                # one exp over both packed blocks
                nc.scalar.activation(out=pt[:, 0:cols],
                                     in_=s_big[:, 0:cols], func=Exp,
                                     scale=INV_SQRT_HS)
                for (kc, off, w, _pt, pcol) in pair:
                    if kc - 4 * qb >= 0:     # diagonal: one [128,128] tri mask
                        nc.vector.tensor_mul(pt[:, pcol:pcol + 128],
                                             pt[:, pcol:pcol + 128], tri_sb)
                lsum = None
                if p0 + 1 < 4 * qb:          # both blocks full width
                    lsum = ls_pool.tile([128, 512], BF16, tag="ls")
                    nc.vector.tensor_add(lsum, pt[:, 0:512], pt[:, 512:1024])
                    if prev is not None and prev[1] not in (None, "DF"):
                        # quad-fold: merge with the previous full pair's sum
                        # and emit its y-matmuls with no denominator matmul
                        nc.vector.tensor_add(lsum, lsum, prev[1])
                        _emit_yl(prev[0], False)
                        prev = None
                else:
                    lsum = "DF"              # diagonal: fold at group end
                    diag_elems.extend(pair)
                if prev is not None:
                    _emit_yl(*prev)
                prev = (pair, lsum)
            _emit_yl(*prev)
            # fold the 4 diagonal blocks query-aligned in fp32, one matmul
            df = ls_pool.tile([128, 512], F32, tag="df")
            for n_, (kc, off, w, pt, pcol) in enumerate(diag_elems):
                if n_ == 0:
                    nc.vector.tensor_copy(out=df, in_=pt[:, pcol:pcol + w])
                else:
                    nc.vector.tensor_add(df[:, off:512], df[:, off:512],
                                         pt[:, pcol:pcol + w])
            df16 = ls_pool.tile([128, 512], BF16, tag="df16")
            nc.vector.tensor_copy(out=df16, in_=df)
            nc.tensor.matmul(l_ps, ones_sb, df16,
                             start=(not l_started[0]), stop=True,
                             skip_group_check=True)
            pending = (h, qb, y_ps, l_ps)
            # interleave one c_proj T-chunk of the PREVIOUS query block
            if qb > 0:
                _cproj_tm((qb - 1) * 4 + h)
    _attn_epilogue(*pending)
    for h in range(QH):
        _cproj_tm(12 + h, tail=(h >= 2))

    ph2.close()
    persist.close()


# ---------------------------------------------------------------- host side

def _rope_cache_np(seq_len, dim):
    inv_freq = 1.0 / (SCALE * BASE ** (np.arange(0, dim, 2, dtype=np.float32) / dim))
    t = np.arange(seq_len, dtype=np.float32)
    freqs = np.outer(t, inv_freq).astype(np.float32)
    emb = np.concatenate([freqs, freqs], axis=-1)
    return np.cos(emb).astype(np.float32), np.sin(emb).astype(np.float32)


_CACHE = {}


def _get_nc():
    if "nc" not in _CACHE:
        _CACHE["nc"] = _build_nc()
    return _CACHE["nc"]


def _f16(a):
    return np.ascontiguousarray(a.astype(np.float16))


def _bf(a):
    return np.ascontiguousarray(a.astype(ml_dtypes.bfloat16))


def kernel(q_x, Wq, bq, Wk, bk, Wv, bv, Wc, bc, _trace=False):
    q_x = np.asarray(q_x, dtype=np.float32)
    Wq = np.asarray(Wq, dtype=np.float32)
    Wk = np.asarray(Wk, dtype=np.float32)
    Wv = np.asarray(Wv, dtype=np.float32)
    Wc = np.asarray(Wc, dtype=np.float32)
    bq = np.asarray(bq, dtype=np.float32)
    bv = np.asarray(bv, dtype=np.float32)
    bc = np.asarray(bc, dtype=np.float32)
    # NOTE: bk is exactly softmax-invariant (adds a per-query constant to all
    # scores) so it is dropped on device.

    x = q_x.reshape(T, C)
    xt = _f16(x.T)                                       # [C, T] fp16

    cos, sin = _rope_cache_np(T, HS)                     # [T, 128]
    cosT = _f16(cos.T)                                   # [128, T]
    sinT = _f16(sin.T)

    ii = np.arange(128)
    tri = _bf((ii[:, None] <= ii[None, :]).astype(np.float32))
    ones = _bf(np.ones((128, 128), dtype=np.float32))
    ident = _bf(np.eye(128, dtype=np.float32))
    rmat = np.zeros((128, 128), dtype=np.float32)        # lhsT of rotate-half
    rmat[ii[:64] + 64, ii[:64]] = -1.0
    rmat[ii[:64], ii[:64] + 64] = 1.0
    rmat = _f16(rmat)

    in_maps = []
    for c in range(NCORES):
        wq_c = Wq[c * DQ:(c + 1) * DQ, :]                # [512, C]
        wk_c = Wk[c * HS:(c + 1) * HS, :]                # [128, C]
        wv_c = Wv[c * HS:(c + 1) * HS, :]
        wqkv = _f16(np.concatenate([wq_c, wk_c, wv_c], axis=0).T)  # [C, 768]
        wct_c = _f16(Wc[:, c * DQ:(c + 1) * DQ].T)       # [512, C]
        bcol = np.zeros((128, QH + 2), dtype=np.float32)
        bcol[:, :QH] = bq[c * DQ:(c + 1) * DQ].reshape(QH, 128).T
        bcol[:, QH] = bv[c * HS:(c + 1) * HS]
        bcol[:, QH + 1] = EXP_BIAS
        in_maps.append({
            "xt": xt, "wqkv": wqkv, "wct": wct_c, "cosT": cosT, "sinT": sinT,
            "tri": tri, "ones": ones, "ident": ident, "rmat": rmat,
            "bcol": bcol,
        })

    nc = _get_nc()
    res = run_bass_kernel_spmd(nc, in_maps, core_ids=list(range(NCORES)),
                               trace=_trace)
    acc = np.zeros((T, C), dtype=np.float64)
    for c in range(NCORES):
        acc += res.results[c]["out"].astype(np.float64)
    out = (acc + bc.astype(np.float64)).astype(np.float32)
    if _trace:
        _CACHE["last_exec_time_ns"] = res.exec_time_ns
        _CACHE["last_results"] = res
    return out.reshape(B, T, C)
